# revision 47
# baseline (speedup 1.0000x reference)
# Multi-head attention (B=4, L=2048, D=512, H=8, dh=64) on 8 trn2 cores.
# Sharding: core c -> batch b = c//2, head-group hg = c%2 (4 heads, 256 out
# channels). Host marshalling: q/k/w cast to bf16; q/wq feature columns
# permuted so the residual slice is columns 0:256; wq/wk rows permuted to
# (i, h, k') order so the projection emits the DoubleRow head-shuffled
# layout directly.
#
# Per-core plan:
#   - bf16 loads; DMA XBAR transposes -> qT/kT/wT (no PE transposes, no
#     casts).
#   - QT/KT projections (bf16): PSUM partition h*32+k' for sub-row i,
#     copy-cast fp8e4m3 on Act -> QT8/KT8 [128, 2*2048].
#   - S^T = K_h Q_h^T as fp8 DoubleRow matmuls (contraction 64 = 32x2).
#   - exp: Act exact exp->bf16; DVE Schraudolph int16 bitcast bf16 (softmax
#     ratio cancels the common-mode scale). 19:13 Act:DVE split.
#   - PV natural: lhsT = P^T chunk [128k,128q], rhs = V+ones [128,65] ->
#     PSUM [128q, 65] accumulated over 16 key blocks; col 64 = denominator.
#     Normalize+residual fused on DVE; batched store per q-block.
import sys

import numpy as np

sys.path.insert(0, "/opt/trn_rl_repo")

L = 2048
D = 512
NH = 4          # heads per core
DH = 64
DHG = NH * DH   # 256 output channels per core
NLT = L // 128  # 16 row tiles
NCI = D // 128  # 4 feature chunks
QB = 512        # q block
NQB = L // QB   # 4
INV_SCALE = 1.0 / float(np.sqrt(D))
# Schraudolph exp in bf16-int space: int16 = round(x*128/ln2 + 16255.4)
SCH_C1 = float(128.0 / np.log(2.0) * INV_SCALE)
SCH_C2 = float(127.0 * 128.0 - 0.6)

_cache = {}


def _build():
    import concourse.bacc as bacc
    import concourse.mybir as mybir
    import concourse.tile as tile

    f32 = mybir.dt.float32
    bf16 = mybir.dt.bfloat16
    i16 = mybir.dt.int16
    fp8 = mybir.dt.float8e4
    EXP = mybir.ActivationFunctionType.Exp
    CPY = mybir.ActivationFunctionType.Copy
    MUL = mybir.AluOpType.mult
    ADD = mybir.AluOpType.add
    DR = mybir.MatmulPerfMode.DoubleRow

    nc = bacc.Bacc("TRN2", target_bir_lowering=False, debug=False, num_devices=8)
    q_d = nc.dram_tensor("q", [L, D], bf16, kind="ExternalInput").ap()
    k_d = nc.dram_tensor("k", [L, D], bf16, kind="ExternalInput").ap()
    wq_d = nc.dram_tensor("wq", [DHG, D], bf16, kind="ExternalInput").ap()
    wk_d = nc.dram_tensor("wk", [DHG, D], bf16, kind="ExternalInput").ap()
    wv_d = nc.dram_tensor("wv", [DHG, D], bf16, kind="ExternalInput").ap()
    o_d = nc.dram_tensor("o", [L, DHG], f32, kind="ExternalOutput").ap()

    with tile.TileContext(nc) as tc:
        with (
            tc.tile_pool(name="static", bufs=1) as st_pool,
            tc.tile_pool(name="ppool", bufs=8) as p_pool,
            tc.tile_pool(name="outsb", bufs=3) as out_pool,
            tc.tile_pool(name="recip", bufs=5) as r_pool,
        ):
            # ---- static tiles (all bf16)
            qres = st_pool.tile([128, NLT * DHG], bf16, name="qres")
            # transposed, chunk-contiguous: cols [c*L, (c+1)*L) = feature chunk c
            qT = st_pool.tile([128, NCI * L], bf16, name="qT")
            kT = st_pool.tile([128, NCI * L], bf16, name="kT")
            wT = [st_pool.tile([128, NCI * DHG], bf16, name=f"wT{x}") for x in range(3)]
            QT8 = st_pool.tile([128, 2 * L], fp8, name="QT8")
            KT8 = st_pool.tile([128, 2 * L], fp8, name="KT8")
            V_big = st_pool.tile([128, NLT * NH * (DH + 1)], bf16, name="Vbig")

            # ---- DRAM-direct XBAR transposes (bf16) + qres slice load
            def w_transpose(x):
                w_d = (wq_d, wk_d, wv_d)[x]
                nc.sync.dma_start_transpose(
                    wT[x].rearrange("p (c o) -> p c o", c=NCI), w_d
                )

            ones_view = V_big.rearrange(
                "p (kt h x) -> p kt h x", kt=NLT, h=NH
            )[:, :, :, DH : DH + 1]
            nc.gpsimd.memset(ones_view, 1.0)

            def q_transpose(lb):
                nc.sync.dma_start_transpose(
                    qT.rearrange("p (c l) -> p c l", c=NCI)[
                        :, :, lb * QB : (lb + 1) * QB
                    ],
                    q_d[lb * QB : (lb + 1) * QB, :],
                )

            def k_transpose(lb):
                nc.sync.dma_start_transpose(
                    kT.rearrange("p (c l) -> p c l", c=NCI)[
                        :, :, lb * QB : (lb + 1) * QB
                    ],
                    k_d[lb * QB : (lb + 1) * QB, :],
                )

            w_transpose(0)
            q_transpose(0)
            w_transpose(1)
            k_transpose(0)
            w_transpose(2)
            for lb in range(1, NQB):
                k_transpose(lb)
            for lb in range(1, NQB):
                q_transpose(lb)
            nc.sync.dma_start(
                out=qres.rearrange("p (lt d) -> p lt d", lt=NLT),
                in_=q_d.rearrange("(lt p) d -> p lt d", p=128)[:, :, 0:DHG],
            )

            def proj_one(x, tT, dst8, lb, i, eng):
                ps = ps_k.tile([128, 512], f32, tag="s1", name="pjps")
                for ci in range(NCI):
                    nc.tensor.matmul(
                        ps,
                        lhsT=wT[x][:, ci * DHG + i * 128 : ci * DHG + (i + 1) * 128],
                        rhs=tT[:, ci * L + lb * QB : ci * L + (lb + 1) * QB],
                        start=(ci == 0),
                        stop=(ci == NCI - 1),
                    )
                dst = dst8[:, i * L + lb * QB : i * L + (lb + 1) * QB]
                if eng == "act":
                    nc.scalar.activation(dst, ps, CPY)
                else:
                    nc.vector.tensor_copy(dst, ps)

            def proj_pair(x, tT, dst8, lb, eng):
                proj_one(x, tT, dst8, lb, 0, eng)
                proj_one(x, tT, dst8, lb, 1, eng)

            def v_proj_duo(lb, half, eng):
                # 2 key tiles in one [128, 512] psum tile (shared bank zero
                # region: start only on the first chain)
                ps = ps_k.tile([128, 512], f32, tag="s1", name="vps")
                for j in range(2):
                    kt = lb * 4 + half * 2 + j
                    for ci in range(NCI):
                        nc.tensor.matmul(
                            ps[:, j * DHG : (j + 1) * DHG],
                            lhsT=kT[:, ci * L + kt * 128 : ci * L + (kt + 1) * 128],
                            rhs=wT[2][:, ci * DHG : (ci + 1) * DHG],
                            start=(ci == 0 and j == 0),
                            stop=(ci == NCI - 1 and j == 1),
                            skip_group_check=True,
                        )
                kt0 = lb * 4 + half * 2
                dst = V_big.rearrange("p (kt h x) -> p kt h x", kt=NLT, h=NH)[
                    :, kt0 : kt0 + 2, :, 0:DH
                ]
                srcv = ps.rearrange("p (j h x) -> p j h x", j=2, h=NH)
                if eng == "act":
                    nc.scalar.activation(dst, srcv, CPY)
                else:
                    nc.vector.tensor_copy(dst, srcv)

            def v_proj_quad(lb, eng):
                v_proj_duo(lb, 0, eng)
                v_proj_duo(lb, 1, eng)

            def dr_views(t8, h):
                return t8[32 * h : 32 * (h + 1), :].rearrange("p (i l) -> p i l", i=2)

            def s_mm(h, kt, qb, out_ap):
                nc.tensor.matmul(
                    out_ap,
                    lhsT=dr_views(KT8, h)[:, :, kt * 128 : (kt + 1) * 128],
                    rhs=dr_views(QT8, h)[:, :, qb * QB : (qb + 1) * QB],
                    start=True,
                    stop=True,
                    perf_mode=DR,
                    tile_position=(32 * h, 0),
                )

            def exp_to(p_dst, s_src, eng):
                if eng == "act":
                    nc.scalar.activation(p_dst, s_src, EXP, scale=INV_SCALE)
                else:
                    nc.vector.tensor_scalar(
                        p_dst.bitcast(i16), s_src, SCH_C1, SCH_C2, MUL, ADD
                    )

            def pv_mm(pv_t, h, kt, p_ap, qsb, first, last):
                nc.tensor.matmul(
                    pv_t[:, qsb * (DH + 1) : (qsb + 1) * (DH + 1)],
                    lhsT=p_ap[:, qsb * 128 : (qsb + 1) * 128],
                    rhs=V_big[:, kt * NH * (DH + 1) + h * (DH + 1) : kt * NH * (DH + 1) + (h + 1) * (DH + 1)],
                    start=first,
                    stop=last,
                    skip_group_check=True,
                )

            def finish_unit(h, qb, pv_t, out_t):
                rc = r_pool.tile([128, 4], f32, tag="rc", name="rc")
                sums = pv_t[:, 0 : 4 * (DH + 1)].rearrange(
                    "p (q x) -> p q x", x=DH + 1
                )[:, :, DH]
                nc.vector.reciprocal(rc, sums)
                for qsb in range(4):
                    nc.vector.scalar_tensor_tensor(
                        out=out_t[:, qsb * DHG + h * DH : qsb * DHG + (h + 1) * DH],
                        in0=pv_t[:, qsb * (DH + 1) : qsb * (DH + 1) + DH],
                        scalar=rc[:, qsb : qsb + 1],
                        in1=qres[:, (qb * 4 + qsb) * DHG + h * DH : (qb * 4 + qsb) * DHG + (h + 1) * DH],
                        op0=MUL,
                        op1=ADD,
                    )

            def dma_out_unit(qb, h, out_t):
                nc.sync.dma_start(
                    out=o_d[qb * QB : (qb + 1) * QB, :].rearrange(
                        "(qsb p) d -> p qsb d", p=128
                    )[:, :, h * DH : (h + 1) * DH],
                    in_=out_t.rearrange("p (qsb d) -> p qsb d", qsb=4)[
                        :, :, h * DH : (h + 1) * DH
                    ],
                )

            # ---- shared pv pool (2 banks) across both phases
            with tc.tile_pool(name="ps_pv2", bufs=2, space="PSUM") as ps_pv:
                unit_pv = {}
                out_ts = {}
                pending = []

                def new_unit(qb, h):
                    if h == 0 or (qb, h) == (0, 2):
                        pass
                    if qb not in out_ts:
                        out_ts[qb] = out_pool.tile(
                            [128, 4 * DHG], f32, tag="ot", name=f"ot{qb}"
                        )
                    unit_pv[(qb, h)] = ps_pv.tile(
                        [128, 512], f32, tag="pv", name="pvq"
                    )

                def drain_one():
                    p_prev, dqb, dh, dpr = pending.pop(0)
                    dpv = unit_pv[(dqb, dh)]
                    for half in range(2):
                        for qsb in range(4):
                            pv_mm(
                                dpv, dh, 2 * dpr + half,
                                p_prev[:, half * 512 : (half + 1) * 512],
                                qsb,
                                first=(dpr == 0 and half == 0 and qsb == 0),
                                last=(dpr == 7 and half == 1 and qsb == 3),
                            )
                    if dpr == 7:
                        finish_unit(dh, dqb, dpv, out_ts[dqb])
                        dma_out_unit(dqb, dh, out_ts[dqb])

                def attn_step(s_pool, qb, h, pr, eng, lag=2):
                    if pr == 0:
                        new_unit(qb, h)
                    s_t = s_pool.tile([128, 1024], f32, tag="sbig", name="sbig")
                    s_mm(h, 2 * pr, qb, s_t[:, 0:512])
                    s_mm(h, 2 * pr + 1, qb, s_t[:, 512:1024])
                    if len(pending) >= lag:
                        drain_one()
                    p_t = p_pool.tile([128, 1024], bf16, tag="p1", name="p1")
                    exp_to(p_t, s_t, eng)
                    pending.append((p_t, qb, h, pr))

                # ---- phase 1: production interleaved with qb0 units h0, h1
                with (
                    tc.tile_pool(name="ps_prod", bufs=2, space="PSUM") as ps_k,
                    tc.tile_pool(name="ps_sA", bufs=2, space="PSUM") as ps_sA,
                ):
                    # PE warmup: ramp the p-state before real inputs arrive
                    warm_sb = st_pool.tile([128, 512], bf16, name="warm")
                    nc.gpsimd.memset(warm_sb, 0.25)
                    warm_ps = ps_k.tile([128, 512], f32, tag="s1", name="warmps")
                    for _ in range(9):
                        nc.tensor.matmul(
                            warm_ps[0:2, :],
                            lhsT=warm_sb[:, 0:2],
                            rhs=warm_sb,
                            start=True,
                            stop=True,
                        )
                    proj_pair(0, qT, QT8, 0, "dve")
                    for lb in range(NQB):
                        proj_pair(1, kT, KT8, lb, "dve")
                        for h in range(2):
                            attn_step(ps_sA, 0, h, lb * 2, "act")
                        v_proj_quad(lb, "dve")
                        if lb == 1:
                            proj_pair(0, qT, QT8, 1, "dve")
                        if lb == 3:
                            proj_pair(0, qT, QT8, 2, "dve")
                            proj_pair(0, qT, QT8, 3, "dve")
                        for h in range(2):
                            e1 = "dve" if (h == 1 and lb % 2 == 1) else "act"
                            attn_step(ps_sA, 0, h, lb * 2 + 1, e1)
                    while pending:
                        drain_one()

                # ---- phase 2: remaining units, flattened pipeline
                with tc.tile_pool(name="ps_qb", bufs=3, space="PSUM") as ps_qb:
                    units = [(0, 2), (0, 3)] + [
                        (qb, h) for qb in range(1, NQB) for h in range(NH)
                    ]
                    steps = []
                    for ui in range(0, len(units), 2):
                        ua, ub = units[ui], units[ui + 1]
                        for pr in range(8):
                            steps.append((ua[0], ua[1], pr))
                            steps.append((ub[0], ub[1], pr))
                    eng_of = ["act" if (i * 67) % 112 < 67 else "dve" for i in range(112)]
                    n_st = len(steps)
                    for si, (qb, h, pr) in enumerate(steps):
                        lg = 3 if si < n_st - 3 else 1
                        attn_step(ps_qb, qb, h, pr, eng_of[si % 112], lag=lg)
                    while pending:
                        drain_one()

    nc.compile()
    return nc


def kernel(query, keys, Wq, Wk, Wv):
    import ml_dtypes

    from concourse.bass_utils import run_bass_kernel_spmd

    if "nc" not in _cache:
        _cache["nc"] = _build()
    nc = _cache["nc"]

    query = np.asarray(query, dtype=np.float32)
    keys = np.asarray(keys, dtype=np.float32)
    Wq = np.asarray(Wq, dtype=np.float32)
    Wk = np.asarray(Wk, dtype=np.float32)
    Wv = np.asarray(Wv, dtype=np.float32)
    B = query.shape[0]
    assert query.shape == (4, L, D) and keys.shape == (4, L, D)
    assert Wq.shape == (D, D) and Wk.shape == (D, D) and Wv.shape == (D, D)

    bf = ml_dtypes.bfloat16
    in_maps = []
    for c in range(8):
        b, hg = c // 2, c % 2
        sl = slice(hg * DHG, (hg + 1) * DHG)
        # permute q/wq feature columns so this core's residual channels are
        # columns 0:256 on device (Q = q @ Wq^T invariant to column perm)
        perm = np.r_[hg * DHG : (hg + 1) * DHG, (1 - hg) * DHG : (2 - hg) * DHG]
        # permute wq/wk ROWS to (i, h, k') order so the projection emits the
        # DoubleRow head-shuffled layout with contiguous weight slices:
        # device row i*128 + h*32 + k'  <-  channel h*64 + i*32 + k'
        rperm = np.array(
            [h * 64 + i * 32 + kk for i in range(2) for h in range(NH) for kk in range(32)]
        )
        in_maps.append(
            {
                "q": np.ascontiguousarray(query[b][:, perm].astype(bf)),
                "k": np.ascontiguousarray(keys[b].astype(bf)),
                "wq": np.ascontiguousarray(Wq[sl][:, perm][rperm].astype(bf)),
                "wk": np.ascontiguousarray(Wk[sl][rperm].astype(bf)),
                "wv": np.ascontiguousarray(Wv[sl].astype(bf)),
            }
        )
    res = run_bass_kernel_spmd(nc, in_maps, list(range(8)), **_cache.get("run_kwargs", {}))
    _cache["last_result"] = res
    out = np.empty((B, L, D), np.float32)
    for c in range(8):
        b, hg = c // 2, c % 2
        out[b][:, hg * DHG : (hg + 1) * DHG] = res.results[c]["o"]
    return out


# revision 48
# speedup vs baseline: 1.0426x; 1.0426x over previous
# Multi-head attention (B=4, L=2048, D=512, H=8, dh=64) on 8 trn2 cores.
# Sharding: core c -> batch b = c//2, head-group hg = c%2 (4 heads, 256 out
# channels). Host marshalling: q/k/w cast to bf16; q/wq feature columns
# permuted so the residual slice is columns 0:256; wq/wk rows permuted to
# (i, h, k') order so the projection emits the DoubleRow head-shuffled
# layout directly.
#
# Per-core plan:
#   - bf16 loads; DMA XBAR transposes -> qT/kT/wT (no PE transposes, no
#     casts).
#   - QT/KT projections (bf16): PSUM partition h*32+k' for sub-row i,
#     copy-cast fp8e4m3 on Act -> QT8/KT8 [128, 2*2048].
#   - S^T = K_h Q_h^T as fp8 DoubleRow matmuls (contraction 64 = 32x2).
#   - exp: Act exact exp->bf16; DVE Schraudolph int16 bitcast bf16 (softmax
#     ratio cancels the common-mode scale). 19:13 Act:DVE split.
#   - PV natural: lhsT = P^T chunk [128k,128q], rhs = V+ones [128,65] ->
#     PSUM [128q, 65] accumulated over 16 key blocks; col 64 = denominator.
#     Normalize+residual fused on DVE; batched store per q-block.
import sys

import numpy as np

sys.path.insert(0, "/opt/trn_rl_repo")

L = 2048
D = 512
NH = 4          # heads per core
DH = 64
DHG = NH * DH   # 256 output channels per core
NLT = L // 128  # 16 row tiles
NCI = D // 128  # 4 feature chunks
QB = 512        # q block
NQB = L // QB   # 4
INV_SCALE = 1.0 / float(np.sqrt(D))
# Schraudolph exp in bf16-int space: int16 = round(x*128/ln2 + 16255.4)
SCH_C1 = float(128.0 / np.log(2.0) * INV_SCALE)
SCH_C2 = float(127.0 * 128.0 - 0.6)

_cache = {}


def _build():
    import concourse.bacc as bacc
    import concourse.mybir as mybir
    import concourse.tile as tile

    f32 = mybir.dt.float32
    bf16 = mybir.dt.bfloat16
    i16 = mybir.dt.int16
    fp8 = mybir.dt.float8e4
    EXP = mybir.ActivationFunctionType.Exp
    CPY = mybir.ActivationFunctionType.Copy
    MUL = mybir.AluOpType.mult
    ADD = mybir.AluOpType.add
    DR = mybir.MatmulPerfMode.DoubleRow

    nc = bacc.Bacc("TRN2", target_bir_lowering=False, debug=False, num_devices=8)
    q_d = nc.dram_tensor("q", [L, D], bf16, kind="ExternalInput").ap()
    k_d = nc.dram_tensor("k", [L, D], bf16, kind="ExternalInput").ap()
    wq_d = nc.dram_tensor("wq", [DHG, D], bf16, kind="ExternalInput").ap()
    wk_d = nc.dram_tensor("wk", [DHG, D], bf16, kind="ExternalInput").ap()
    wv_d = nc.dram_tensor("wv", [DHG, D], bf16, kind="ExternalInput").ap()
    o_d = nc.dram_tensor("o", [L, DHG], f32, kind="ExternalOutput").ap()

    with tile.TileContext(nc) as tc:
        with (
            tc.tile_pool(name="static", bufs=1) as st_pool,
            tc.tile_pool(name="ppool", bufs=8) as p_pool,
            tc.tile_pool(name="outsb", bufs=3) as out_pool,
            tc.tile_pool(name="recip", bufs=5) as r_pool,
        ):
            # ---- static tiles (all bf16)
            qres = st_pool.tile([128, NLT * DHG], bf16, name="qres")
            # transposed, chunk-contiguous: cols [c*L, (c+1)*L) = feature chunk c
            qT = st_pool.tile([128, NCI * L], bf16, name="qT")
            kT = st_pool.tile([128, NCI * L], bf16, name="kT")
            wT = [st_pool.tile([128, NCI * DHG], bf16, name=f"wT{x}") for x in range(3)]
            QT8 = st_pool.tile([128, 2 * L], fp8, name="QT8")
            KT8 = st_pool.tile([128, 2 * L], fp8, name="KT8")
            V_big = st_pool.tile([128, NLT * NH * (DH + 1)], bf16, name="Vbig")

            # ---- DRAM-direct XBAR transposes (bf16) + qres slice load
            def w_transpose(x):
                w_d = (wq_d, wk_d, wv_d)[x]
                nc.sync.dma_start_transpose(
                    wT[x].rearrange("p (c o) -> p c o", c=NCI), w_d
                )

            ones_view = V_big.rearrange(
                "p (kt h x) -> p kt h x", kt=NLT, h=NH
            )[:, :, :, DH : DH + 1]
            nc.gpsimd.memset(ones_view, 1.0)

            def q_transpose(lb):
                nc.sync.dma_start_transpose(
                    qT.rearrange("p (c l) -> p c l", c=NCI)[
                        :, :, lb * QB : (lb + 1) * QB
                    ],
                    q_d[lb * QB : (lb + 1) * QB, :],
                )

            def k_transpose(lb):
                nc.sync.dma_start_transpose(
                    kT.rearrange("p (c l) -> p c l", c=NCI)[
                        :, :, lb * QB : (lb + 1) * QB
                    ],
                    k_d[lb * QB : (lb + 1) * QB, :],
                )

            w_transpose(0)
            q_transpose(0)
            w_transpose(1)
            k_transpose(0)
            w_transpose(2)
            for lb in range(1, NQB):
                k_transpose(lb)
            for lb in range(1, NQB):
                q_transpose(lb)
            nc.sync.dma_start(
                out=qres.rearrange("p (lt d) -> p lt d", lt=NLT),
                in_=q_d.rearrange("(lt p) d -> p lt d", p=128)[:, :, 0:DHG],
            )

            def proj_one(x, tT, dst8, lb, i, eng):
                ps = ps_k.tile([128, 512], f32, tag="s1", name="pjps")
                for ci in range(NCI):
                    nc.tensor.matmul(
                        ps,
                        lhsT=wT[x][:, ci * DHG + i * 128 : ci * DHG + (i + 1) * 128],
                        rhs=tT[:, ci * L + lb * QB : ci * L + (lb + 1) * QB],
                        start=(ci == 0),
                        stop=(ci == NCI - 1),
                    )
                dst = dst8[:, i * L + lb * QB : i * L + (lb + 1) * QB]
                if eng == "act":
                    nc.scalar.activation(dst, ps, CPY)
                else:
                    nc.vector.tensor_copy(dst, ps)

            def proj_pair(x, tT, dst8, lb, eng):
                proj_one(x, tT, dst8, lb, 0, eng)
                proj_one(x, tT, dst8, lb, 1, eng)

            def v_proj_duo(lb, half, eng):
                # 2 key tiles in one [128, 512] psum tile (shared bank zero
                # region: start only on the first chain)
                ps = ps_k.tile([128, 512], f32, tag="s1", name="vps")
                for j in range(2):
                    kt = lb * 4 + half * 2 + j
                    for ci in range(NCI):
                        nc.tensor.matmul(
                            ps[:, j * DHG : (j + 1) * DHG],
                            lhsT=kT[:, ci * L + kt * 128 : ci * L + (kt + 1) * 128],
                            rhs=wT[2][:, ci * DHG : (ci + 1) * DHG],
                            start=(ci == 0 and j == 0),
                            stop=(ci == NCI - 1 and j == 1),
                            skip_group_check=True,
                        )
                kt0 = lb * 4 + half * 2
                dst = V_big.rearrange("p (kt h x) -> p kt h x", kt=NLT, h=NH)[
                    :, kt0 : kt0 + 2, :, 0:DH
                ]
                srcv = ps.rearrange("p (j h x) -> p j h x", j=2, h=NH)
                if eng == "act":
                    nc.scalar.activation(dst, srcv, CPY)
                else:
                    nc.vector.tensor_copy(dst, srcv)

            def v_proj_quad(lb, eng):
                v_proj_duo(lb, 0, eng)
                v_proj_duo(lb, 1, eng)

            def dr_views(t8, h):
                return t8[32 * h : 32 * (h + 1), :].rearrange("p (i l) -> p i l", i=2)

            def s_mm(h, kt, qb, out_ap):
                nc.tensor.matmul(
                    out_ap,
                    lhsT=dr_views(KT8, h)[:, :, kt * 128 : (kt + 1) * 128],
                    rhs=dr_views(QT8, h)[:, :, qb * QB : (qb + 1) * QB],
                    start=True,
                    stop=True,
                    perf_mode=DR,
                    tile_position=(32 * h, 0),
                )

            def exp_to(p_dst, s_src, eng):
                if eng == "act":
                    nc.scalar.activation(p_dst, s_src, EXP, scale=INV_SCALE)
                else:
                    nc.vector.tensor_scalar(
                        p_dst.bitcast(i16), s_src, SCH_C1, SCH_C2, MUL, ADD
                    )

            def pv_mm(pv_t, h, kt, p_ap, qsb, first, last):
                nc.tensor.matmul(
                    pv_t[:, qsb * (DH + 1) : (qsb + 1) * (DH + 1)],
                    lhsT=p_ap[:, qsb * 128 : (qsb + 1) * 128],
                    rhs=V_big[:, kt * NH * (DH + 1) + h * (DH + 1) : kt * NH * (DH + 1) + (h + 1) * (DH + 1)],
                    start=first,
                    stop=last,
                    skip_group_check=True,
                )

            def finish_unit(h, qb, pv_t, out_t):
                rc = r_pool.tile([128, 4], f32, tag="rc", name="rc")
                sums = pv_t[:, 0 : 4 * (DH + 1)].rearrange(
                    "p (q x) -> p q x", x=DH + 1
                )[:, :, DH]
                nc.vector.reciprocal(rc, sums)
                for qsb in range(4):
                    nc.vector.scalar_tensor_tensor(
                        out=out_t[:, qsb * DHG + h * DH : qsb * DHG + (h + 1) * DH],
                        in0=pv_t[:, qsb * (DH + 1) : qsb * (DH + 1) + DH],
                        scalar=rc[:, qsb : qsb + 1],
                        in1=qres[:, (qb * 4 + qsb) * DHG + h * DH : (qb * 4 + qsb) * DHG + (h + 1) * DH],
                        op0=MUL,
                        op1=ADD,
                    )

            def dma_out_unit(qb, h, out_t):
                nc.sync.dma_start(
                    out=o_d[qb * QB : (qb + 1) * QB, :].rearrange(
                        "(qsb p) d -> p qsb d", p=128
                    )[:, :, h * DH : (h + 1) * DH],
                    in_=out_t.rearrange("p (qsb d) -> p qsb d", qsb=4)[
                        :, :, h * DH : (h + 1) * DH
                    ],
                )

            # ---- shared pv pool (2 banks) across both phases
            with tc.tile_pool(name="ps_pv2", bufs=2, space="PSUM") as ps_pv:
                unit_pv = {}
                out_ts = {}
                pending = []

                def new_unit(qb, h):
                    if h == 0 or (qb, h) == (0, 2):
                        pass
                    if qb not in out_ts:
                        out_ts[qb] = out_pool.tile(
                            [128, 4 * DHG], f32, tag="ot", name=f"ot{qb}"
                        )
                    unit_pv[(qb, h)] = ps_pv.tile(
                        [128, 512], f32, tag="pv", name="pvq"
                    )

                def drain_one():
                    p_prev, dqb, dh, dpr = pending.pop(0)
                    dpv = unit_pv[(dqb, dh)]
                    for half in range(2):
                        for qsb in range(4):
                            pv_mm(
                                dpv, dh, 2 * dpr + half,
                                p_prev[:, half * 512 : (half + 1) * 512],
                                qsb,
                                first=(dpr == 0 and half == 0 and qsb == 0),
                                last=(dpr == 7 and half == 1 and qsb == 3),
                            )
                    if dpr == 7:
                        finish_unit(dh, dqb, dpv, out_ts[dqb])
                        dma_out_unit(dqb, dh, out_ts[dqb])

                def attn_step(s_pool, qb, h, pr, eng, lag=2):
                    if pr == 0:
                        new_unit(qb, h)
                    s_t = s_pool.tile([128, 1024], f32, tag="sbig", name="sbig")
                    s_mm(h, 2 * pr, qb, s_t[:, 0:512])
                    s_mm(h, 2 * pr + 1, qb, s_t[:, 512:1024])
                    if len(pending) >= lag:
                        drain_one()
                    p_t = p_pool.tile([128, 1024], bf16, tag="p1", name="p1")
                    exp_to(p_t, s_t, eng)
                    pending.append((p_t, qb, h, pr))

                # ---- phase 1: production interleaved with qb0 units h0, h1
                with (
                    tc.tile_pool(name="ps_prod", bufs=2, space="PSUM") as ps_k,
                    tc.tile_pool(name="ps_sA", bufs=2, space="PSUM") as ps_sA,
                ):
                    # PE warmup: ramp the p-state before real inputs arrive
                    warm_sb = st_pool.tile([128, 512], bf16, name="warm")
                    nc.gpsimd.memset(warm_sb, 0.25)
                    warm_ps = ps_k.tile([128, 512], f32, tag="s1", name="warmps")
                    for _ in range(9):
                        nc.tensor.matmul(
                            warm_ps[0:2, :],
                            lhsT=warm_sb[:, 0:2],
                            rhs=warm_sb,
                            start=True,
                            stop=True,
                        )
                    proj_pair(0, qT, QT8, 0, "dve")
                    for lb in range(NQB):
                        proj_pair(1, kT, KT8, lb, "dve")
                        for h in range(2):
                            attn_step(ps_sA, 0, h, lb * 2, "act")
                        v_proj_quad(lb, "dve")
                        if lb == 1:
                            proj_pair(0, qT, QT8, 1, "dve")
                        if lb == 3:
                            proj_pair(0, qT, QT8, 2, "dve")
                            proj_pair(0, qT, QT8, 3, "dve")
                        for h in range(2):
                            e1 = "dve" if (h == 1 and lb % 2 == 1) else "act"
                            attn_step(ps_sA, 0, h, lb * 2 + 1, e1)
                    while pending:
                        drain_one()

                # ---- phase 2: remaining units, flattened pipeline
                with tc.tile_pool(name="ps_qb", bufs=3, space="PSUM") as ps_qb:
                    units = [(0, 2), (0, 3)] + [
                        (qb, h) for qb in range(1, NQB) for h in range(NH)
                    ]
                    steps = [(qb, h, pr) for qb, h in units for pr in range(8)]
                    eng_of = ["act" if (i * 67) % 112 < 67 else "dve" for i in range(112)]
                    n_st = len(steps)
                    for si, (qb, h, pr) in enumerate(steps):
                        lg = 3 if si < n_st - 3 else 1
                        attn_step(ps_qb, qb, h, pr, eng_of[si % 112], lag=lg)
                    while pending:
                        drain_one()

    nc.compile()
    return nc


def kernel(query, keys, Wq, Wk, Wv):
    import ml_dtypes

    from concourse.bass_utils import run_bass_kernel_spmd

    if "nc" not in _cache:
        _cache["nc"] = _build()
    nc = _cache["nc"]

    query = np.asarray(query, dtype=np.float32)
    keys = np.asarray(keys, dtype=np.float32)
    Wq = np.asarray(Wq, dtype=np.float32)
    Wk = np.asarray(Wk, dtype=np.float32)
    Wv = np.asarray(Wv, dtype=np.float32)
    B = query.shape[0]
    assert query.shape == (4, L, D) and keys.shape == (4, L, D)
    assert Wq.shape == (D, D) and Wk.shape == (D, D) and Wv.shape == (D, D)

    bf = ml_dtypes.bfloat16
    in_maps = []
    for c in range(8):
        b, hg = c // 2, c % 2
        sl = slice(hg * DHG, (hg + 1) * DHG)
        # permute q/wq feature columns so this core's residual channels are
        # columns 0:256 on device (Q = q @ Wq^T invariant to column perm)
        perm = np.r_[hg * DHG : (hg + 1) * DHG, (1 - hg) * DHG : (2 - hg) * DHG]
        # permute wq/wk ROWS to (i, h, k') order so the projection emits the
        # DoubleRow head-shuffled layout with contiguous weight slices:
        # device row i*128 + h*32 + k'  <-  channel h*64 + i*32 + k'
        rperm = np.array(
            [h * 64 + i * 32 + kk for i in range(2) for h in range(NH) for kk in range(32)]
        )
        in_maps.append(
            {
                "q": np.ascontiguousarray(query[b][:, perm].astype(bf)),
                "k": np.ascontiguousarray(keys[b].astype(bf)),
                "wq": np.ascontiguousarray(Wq[sl][:, perm][rperm].astype(bf)),
                "wk": np.ascontiguousarray(Wk[sl][rperm].astype(bf)),
                "wv": np.ascontiguousarray(Wv[sl].astype(bf)),
            }
        )
    res = run_bass_kernel_spmd(nc, in_maps, list(range(8)), **_cache.get("run_kwargs", {}))
    _cache["last_result"] = res
    out = np.empty((B, L, D), np.float32)
    for c in range(8):
        b, hg = c // 2, c % 2
        out[b][:, hg * DHG : (hg + 1) * DHG] = res.results[c]["o"]
    return out
